# revision 21
# baseline (speedup 1.0000x reference)
"""Multi-head attention (B=4, S=2048, D=1024, H=16) on 8 TRN2 NeuronCores.

Sharding (data + head parallel): core c handles batch b = c//2 and head
group g = c%2 (8 of the 16 heads, feature columns 512g:512(g+1)).
Each core computes its heads' full attention locally and a partial
output projection; the host sums the two partials per batch and adds
b_o plus the b_v @ W_o term (softmax rows sum to 1, so the V bias is an
exact constant output offset and never touches the device).

On-device layout is feature-major ("transposed"): activations live as
[feature, seq] so every linear layer is matmul(lhsT=W-block, rhs=x^T)
with W loaded from HBM exactly as stored (in, out).  The host passes
q/k/v pre-transposed per batch and receives the partial output
transposed back.

Pipeline per core (all matmul moving dims 512, bf16 compute with fp32
PSUM accumulation; measured absmax relative error vs the fp32
reference ~5.4e-3):
  V     = x @ Wv (bf16), natural [seq, feat] layout, evacuated with a
          ones column per head (V_aug [j, 8*65])
  KT/QT = (x @ Wk/Wq)^T (bf16) + bias (per-partition) on evacuation
  scores^T[j, i] per head pair via row-packed K=64 matmuls (the two
          heads run concurrently on separate 64-row tile groups),
          softmax exp on ScalarE directly from PSUM ([128,1024] grain,
          scale=1/8 folded in; no max subtraction: scores ~ N(0,1) so
          exp is safely bounded), probabilities written bf16
  PV    = V_aug^T @ P^T accumulated over 16 j-blocks in PSUM -> rows
          0:64 head output (transposed), row 64 softmax denominator.
          PSUM rows are evacuated to SBUF immediately (frees the bank);
          normalization = gpsimd partition_broadcast of the denominator
          + reciprocal_approx_fast + vector multiply, off the critical
          path, bf16 attnT out.
  out   = Wo^T @ attnT (bf16), fp32 partial written to HBM.
"""

import os

import numpy as np

import concourse.bass as bass  # noqa: F401
import concourse.mybir as mybir
import concourse.tile as tile
from concourse import bacc
from concourse.bass_utils import run_bass_kernel_spmd

f32 = mybir.dt.float32
bf16 = mybir.dt.bfloat16
Exp = mybir.ActivationFunctionType.Exp
MULT = mybir.AluOpType.mult

B, S, D = 4, 2048, 1024
H_LOC = 8
DK = 64
DG = 512
KB = D // 128
PB = DG // 128
JB = S // 128
IC = S // 512
N = 512
QK_DT = bf16


def _build():
    nc = bacc.Bacc("TRN2")

    xq = nc.dram_tensor("xq", (D, S), QK_DT, kind="ExternalInput")
    xk = nc.dram_tensor("xk", (D, S), QK_DT, kind="ExternalInput")
    xv = nc.dram_tensor("xv", (D, S), bf16, kind="ExternalInput")
    wq = nc.dram_tensor("wq", (D, DG), QK_DT, kind="ExternalInput")
    wk = nc.dram_tensor("wk", (D, DG), QK_DT, kind="ExternalInput")
    wv = nc.dram_tensor("wv", (D, DG), bf16, kind="ExternalInput")
    wo = nc.dram_tensor("wo", (DG, D), bf16, kind="ExternalInput")
    bq = nc.dram_tensor("bq", (DG,), f32, kind="ExternalInput")
    bk = nc.dram_tensor("bk", (DG,), f32, kind="ExternalInput")
    o_t = nc.dram_tensor("o_t", (D, S), f32, kind="ExternalOutput")

    with tile.TileContext(nc) as tc:
        with (
            tc.tile_pool(name="persist", bufs=1) as persist,
            tc.tile_pool(name="wp", bufs=3) as wp,
            tc.tile_pool(name="xp", bufs=8) as xp,
            tc.tile_pool(name="xvp", bufs=8) as xvp,
            tc.tile_pool(name="ptp", bufs=32) as ptp,
            tc.tile_pool(name="pvs", bufs=2) as pvsp,
            tc.tile_pool(name="rbp", bufs=2) as rbp,
            tc.tile_pool(name="osb", bufs=2) as osbp,
            tc.tile_pool(name="sps", bufs=3, space="PSUM") as sps,
            tc.tile_pool(name="mps", bufs=2, space="PSUM") as mps,
        ):
            # ---- persistent tensors -------------------------------------
            QT = [persist.tile([128, S], QK_DT, tag=f"qt{p}", name=f"qt{p}")
                  for p in range(PB)]
            KT = [persist.tile([128, S], QK_DT, tag=f"kt{p}", name=f"kt{p}")
                  for p in range(PB)]
            VA = [persist.tile([128, H_LOC, DK + 1], bf16, tag=f"va{j}",
                               name=f"va{j}") for j in range(JB)]
            AT = [persist.tile([128, S], bf16, tag=f"at{p}", name=f"at{p}")
                  for p in range(PB)]

            bq_t = persist.tile([128, PB], f32, tag="bq")
            bk_t = persist.tile([128, PB], f32, tag="bk")
            nc.sync.dma_start(out=bq_t, in_=bq.rearrange("(pb p) -> p pb", p=128))
            nc.sync.dma_start(out=bk_t, in_=bk.rearrange("(pb p) -> p pb", p=128))
            for j in range(JB):
                nc.vector.memset(VA[j][:, :, DK:DK + 1], 1.0)

            # ---- V projection (bf16, natural [seq, feature] layout) -----
            def v_proj(wv_t):
              for jg in range(4):
                xc = []
                for kb in range(KB):
                    t = xvp.tile([128, N], bf16, tag="xcv", name="xcv")
                    nc.sync.dma_start(
                        out=t,
                        in_=xv[kb * 128:(kb + 1) * 128, jg * N:(jg + 1) * N],
                    )
                    xc.append(t)
                for jj in range(4):
                    j = jg * 4 + jj
                    ps = mps.tile([128, N], f32, tag="mm", name="vps")
                    for kb in range(KB):
                        nc.tensor.matmul(
                            ps,
                            xc[kb][:, jj * 128:(jj + 1) * 128],
                            wv_t[:, kb, :],
                            start=(kb == 0),
                            stop=(kb == KB - 1),
                        )
                    nc.vector.tensor_copy(
                        VA[j][:, :, 0:DK],
                        ps.rearrange("p (h e) -> p h e", e=DK),
                    )

            # ---- K then Q projections (feature-major output) ------------
            def project_qk(x_dram, w_dram, bias_t, out_tiles, label):
                w_t = wp.tile([128, KB, N], QK_DT, tag="w", name=f"w_{label}")
                nc.sync.dma_start(
                    out=w_t, in_=w_dram.rearrange("(kb p) n -> p kb n", p=128)
                )
                for ic in range(IC):
                    xc = []
                    for kb in range(KB):
                        t = xp.tile([128, N], QK_DT, tag="xc", name=f"xc_{label}")
                        nc.sync.dma_start(
                            out=t,
                            in_=x_dram[kb * 128:(kb + 1) * 128,
                                       ic * N:(ic + 1) * N],
                        )
                        xc.append(t)
                    for pb in range(PB):
                        ps = mps.tile([128, N], f32, tag="mm", name=f"ps_{label}")
                        for kb in range(KB):
                            nc.tensor.matmul(
                                ps,
                                w_t[:, kb, pb * 128:(pb + 1) * 128],
                                xc[kb],
                                start=(kb == 0),
                                stop=(kb == KB - 1),
                            )
                        nc.vector.tensor_scalar_add(
                            out_tiles[pb][:, ic * N:(ic + 1) * N],
                            ps,
                            bias_t[:, pb:pb + 1],
                        )


            # ---- attention, software-pipelined ---------------------------
            # Groups run ic-major.  Emission order per group: scores+exp
            # (high priority - keeps ScalarE fed), then the PREVIOUS
            # group's PV + normalization as PE filler inside the
            # ACT-bound scores window.  Q projection for each i-chunk and
            # the previous i-chunk's output projection are emitted as
            # filler too.

            def q_proj_ic(ic):
                xc = []
                for kb in range(KB):
                    t = xp.tile([128, N], QK_DT, tag="xc", name="xc_q")
                    nc.sync.dma_start(
                        out=t,
                        in_=xq[kb * 128:(kb + 1) * 128, ic * N:(ic + 1) * N],
                    )
                    xc.append(t)
                for pb in range(PB):
                    ps = mps.tile([128, N], f32, tag="mm", name="ps_q")
                    for kb in range(KB):
                        nc.tensor.matmul(
                            ps,
                            wq_t[:, kb, pb * 128:(pb + 1) * 128],
                            xc[kb],
                            start=(kb == 0),
                            stop=(kb == KB - 1),
                        )
                    nc.vector.tensor_scalar_add(
                        QT[pb][:, ic * N:(ic + 1) * N],
                        ps,
                        bq_t[:, pb:pb + 1],
                    )

            def scores_phase(pair, ic):
                pts = []
                for j in range(JB):
                    s_ps = sps.tile([128, 2 * N], f32, tag="s", name="s_ps")
                    nc.tensor.matmul(
                        s_ps[:, 0:N],
                        KT[pair][0:64, j * 128:(j + 1) * 128],
                        QT[pair][0:64, ic * N:(ic + 1) * N],
                        start=True, stop=True,
                    )
                    nc.tensor.matmul(
                        s_ps[:, N:2 * N],
                        KT[pair][64:128, j * 128:(j + 1) * 128],
                        QT[pair][64:128, ic * N:(ic + 1) * N],
                        start=True, stop=True,
                        tile_position=(64, 0),
                    )
                    pt = ptp.tile([128, 2 * N], bf16, tag="pt", name="pt")
                    nc.scalar.activation(pt, s_ps, Exp, scale=0.125)
                    pts.append(pt)
                return pts

            def pv_phase(pair, ic, pts):
                pv = [
                    mps.tile([DK + 1, N], f32, tag="mm", name="pv0"),
                    mps.tile([DK + 1, N], f32, tag="mm", name="pv1"),
                ]
                for h2 in range(2):
                    for j in range(JB):
                        nc.tensor.matmul(
                            pv[h2],
                            VA[j][:, 2 * pair + h2, :],
                            pts[j][:, h2 * N:(h2 + 1) * N],
                            start=(j == 0),
                            stop=(j == JB - 1),
                        )
                for h2 in range(2):
                    pvs = pvsp.tile([DK + 1, N], f32, tag="pvs", name="pvs")
                    nc.vector.tensor_copy(pvs[0:DK, :], pv[h2][0:DK, :])
                    den = rbp.tile([1, N], f32, tag="den", name="den")
                    nc.vector.tensor_copy(den, pv[h2][DK:DK + 1, :])
                    rbr = rbp.tile([64, N], f32, tag="rbr", name="rbr")
                    nc.gpsimd.partition_broadcast(rbr, den)
                    rb = rbp.tile([64, N], f32, tag="rb", name="rb")
                    nc.vector.reciprocal_approx_fast(rb, rbr)
                    dst = AT[pair][h2 * 64:(h2 + 1) * 64, ic * N:(ic + 1) * N]
                    nc.vector.tensor_tensor(
                        out=dst, in0=pvs[0:DK, :], in1=rb, op=MULT
                    )

            def oproj_ic(ic):
                for dob in range(KB):
                    ops = mps.tile([128, N], f32, tag="mm", name="ops")
                    for pb in range(PB):
                        nc.tensor.matmul(
                            ops,
                            wo_t[:, pb, dob * 128:(dob + 1) * 128],
                            AT[pb][:, ic * N:(ic + 1) * N],
                            start=(pb == 0),
                            stop=(pb == PB - 1),
                        )
                    ob = osbp.tile([128, N], f32, tag="ob", name="ob")
                    nc.vector.tensor_copy(ob, ops)
                    nc.sync.dma_start(
                        out=o_t[dob * 128:(dob + 1) * 128, ic * N:(ic + 1) * N],
                        in_=ob,
                    )

            def k_proj_pb(pb):
                # pb-major K projection: completes KT[pb] in one shot so
                # pair pb's attention can start; x chunks re-read per pb
                for ic2 in range(IC):
                    xck = []
                    for kb in range(KB):
                        t = xp.tile([128, N], QK_DT, tag="xc", name="xc_k")
                        nc.sync.dma_start(
                            out=t,
                            in_=xk[kb * 128:(kb + 1) * 128,
                                   ic2 * N:(ic2 + 1) * N],
                        )
                        xck.append(t)
                    ps = mps.tile([128, N], f32, tag="mm", name="ps_k")
                    for kb in range(KB):
                        nc.tensor.matmul(
                            ps,
                            wk_t[:, kb, pb * 128:(pb + 1) * 128],
                            xck[kb],
                            start=(kb == 0),
                            stop=(kb == KB - 1),
                        )
                    nc.vector.tensor_scalar_add(
                        KT[pb][:, ic2 * N:(ic2 + 1) * N],
                        ps,
                        bk_t[:, pb:pb + 1],
                    )

            wk_t = wp.tile([128, KB, N], QK_DT, tag="w", name="w_k")
            nc.sync.dma_start(
                out=wk_t, in_=wk.rearrange("(kb p) n -> p kb n", p=128)
            )
            wv_t = wp.tile([128, KB, N], bf16, tag="w", name="wv_t")
            nc.sync.dma_start(
                out=wv_t, in_=wv.rearrange("(kb p) n -> p kb n", p=128)
            )
            wq_t = wp.tile([128, KB, N], QK_DT, tag="w", name="w_q")
            nc.sync.dma_start(
                out=wq_t, in_=wq.rearrange("(kb p) n -> p kb n", p=128)
            )
            wo_t = wp.tile([128, PB, D], bf16, tag="w", name="wo_t")
            nc.sync.dma_start(
                out=wo_t, in_=wo.rearrange("(pb p) n -> p pb n", p=128)
            )

            k_proj_pb(0)
            q_proj_ic(0)
            v_proj(wv_t)

            prev = None          # (pair, ic, pts) of the unconsumed group
            for ic in range(IC):
                if ic > 0:
                    q_proj_ic(ic)
                for pair in range(PB):
                    pts = scores_phase(pair, ic)
                    if ic == 0 and pair < PB - 1:
                        k_proj_pb(pair + 1)
                    if prev is not None:
                        pv_phase(*prev)
                        if pair == 1 and ic > 0:
                            oproj_ic(ic - 1)
                    prev = (pair, ic, pts)
            pv_phase(*prev)
            oproj_ic(IC - 1)

    nc.compile()
    return nc


_NC_CACHE = None


def _get_nc():
    global _NC_CACHE
    if _NC_CACHE is None:
        _NC_CACHE = _build()
    return _NC_CACHE


def kernel(q, k, v, W_q, b_q, W_k, b_k, W_v, b_v, W_o, b_o):
    import ml_dtypes

    q = np.asarray(q, dtype=np.float32)
    k = np.asarray(k, dtype=np.float32)
    v = np.asarray(v, dtype=np.float32)
    W_q = np.asarray(W_q, dtype=np.float32)
    W_k = np.asarray(W_k, dtype=np.float32)
    W_v = np.asarray(W_v, dtype=np.float32)
    W_o = np.asarray(W_o, dtype=np.float32)
    b_q = np.asarray(b_q, dtype=np.float32)
    b_k = np.asarray(b_k, dtype=np.float32)
    b_v = np.asarray(b_v, dtype=np.float32)
    b_o = np.asarray(b_o, dtype=np.float32)

    bf = ml_dtypes.bfloat16
    xq_t = [np.ascontiguousarray(q[b].T).astype(bf) for b in range(B)]
    xk_t = [np.ascontiguousarray(k[b].T).astype(bf) for b in range(B)]
    xv_t = [np.ascontiguousarray(v[b].T).astype(bf) for b in range(B)]
    wq_s = [np.ascontiguousarray(W_q[:, g * DG:(g + 1) * DG]).astype(bf)
            for g in range(2)]
    wk_s = [np.ascontiguousarray(W_k[:, g * DG:(g + 1) * DG]).astype(bf)
            for g in range(2)]
    wv_s = [np.ascontiguousarray(W_v[:, g * DG:(g + 1) * DG]).astype(bf)
            for g in range(2)]
    wo_s = [np.ascontiguousarray(W_o[g * DG:(g + 1) * DG, :]).astype(bf)
            for g in range(2)]

    in_maps = []
    for c in range(8):
        b, g = c // 2, c % 2
        in_maps.append({
            "xq": xq_t[b], "xk": xk_t[b], "xv": xv_t[b],
            "wq": wq_s[g], "wk": wk_s[g], "wv": wv_s[g], "wo": wo_s[g],
            "bq": b_q[g * DG:(g + 1) * DG],
            "bk": b_k[g * DG:(g + 1) * DG],
        })

    nc = _get_nc()
    trace = bool(int(os.environ.get("KERNEL_TRACE", "0")))
    if trace:
        try:
            import axon_profile_shim
            axon_profile_shim.install()
        except Exception:
            pass
    res = run_bass_kernel_spmd(nc, in_maps, core_ids=list(range(8)), trace=trace)
    if res.exec_time_ns is not None:
        print(f"HW exec time: {res.exec_time_ns} ns", flush=True)

    out = np.empty((B, S, D), dtype=np.float32)
    # b_v is an exact constant output offset: softmax rows sum to 1, so
    # attn @ (V + 1 b_v^T) @ W_o = attn @ V @ W_o + b_v @ W_o.
    bv_off = [b_v[g * DG:(g + 1) * DG] @ W_o[g * DG:(g + 1) * DG, :]
              for g in range(2)]
    full_bias = b_o + bv_off[0] + bv_off[1]
    for b in range(B):
        part = res.results[2 * b]["o_t"] + res.results[2 * b + 1]["o_t"]
        out[b] = part.T + full_bias
    return out


# revision 23
# speedup vs baseline: 1.2160x; 1.2160x over previous
"""Multi-head attention (B=4, S=2048, D=1024, H=16) on 8 TRN2 NeuronCores.

Sharding (data + head parallel): core c handles batch b = c//2 and head
group g = c%2 (8 of the 16 heads, feature columns 512g:512(g+1)).
Each core computes its heads' full attention locally and a partial
output projection; the host sums the two partials per batch and adds
b_o plus the b_v @ W_o term (softmax rows sum to 1, so the V bias is an
exact constant output offset and never touches the device).

On-device layout is feature-major ("transposed"): activations live as
[feature, seq] so every linear layer is matmul(lhsT=W-block, rhs=x^T)
with W loaded from HBM exactly as stored (in, out).  The host passes
q/k/v pre-transposed per batch and receives the partial output
transposed back.

Pipeline per core (all matmul moving dims 512, bf16 compute with fp32
PSUM accumulation; measured absmax relative error vs the fp32
reference ~5.4e-3):
  V     = x @ Wv (bf16), natural [seq, feat] layout, evacuated with a
          ones column per head (V_aug [j, 8*65])
  KT/QT = (x @ Wk/Wq)^T (bf16) + bias (per-partition) on evacuation
  scores^T[j, i] per head pair via row-packed K=64 matmuls (the two
          heads run concurrently on separate 64-row tile groups),
          softmax exp on ScalarE directly from PSUM ([128,1024] grain,
          scale=1/8 folded in; no max subtraction: scores ~ N(0,1) so
          exp is safely bounded), probabilities written bf16
  PV    = V_aug^T @ P^T accumulated over 16 j-blocks in PSUM -> rows
          0:64 head output (transposed), row 64 softmax denominator.
          PSUM rows are evacuated to SBUF immediately (frees the bank);
          normalization = gpsimd partition_broadcast of the denominator
          + reciprocal_approx_fast + vector multiply, off the critical
          path, bf16 attnT out.
  out   = Wo^T @ attnT (bf16), fp32 partial written to HBM.
"""

import os

import numpy as np

import concourse.bass as bass  # noqa: F401
import concourse.mybir as mybir
import concourse.tile as tile
from concourse import bacc
from concourse.bass_utils import run_bass_kernel_spmd

f32 = mybir.dt.float32
bf16 = mybir.dt.bfloat16
Exp = mybir.ActivationFunctionType.Exp
MULT = mybir.AluOpType.mult

B, S, D = 4, 2048, 1024
H_LOC = 8
DK = 64
DG = 512
KB = D // 128
PB = DG // 128
JB = S // 128
IC = S // 512
N = 512
QK_DT = bf16


def _build():
    nc = bacc.Bacc("TRN2")

    xq = nc.dram_tensor("xq", (D, S), QK_DT, kind="ExternalInput")
    xk = nc.dram_tensor("xk", (D, S), QK_DT, kind="ExternalInput")
    xv = nc.dram_tensor("xv", (D, S), bf16, kind="ExternalInput")
    wq = nc.dram_tensor("wq", (D, DG), QK_DT, kind="ExternalInput")
    wk = nc.dram_tensor("wk", (D, DG), QK_DT, kind="ExternalInput")
    wv = nc.dram_tensor("wv", (D, DG), bf16, kind="ExternalInput")
    wo = nc.dram_tensor("wo", (DG, D), bf16, kind="ExternalInput")
    bq = nc.dram_tensor("bq", (DG,), f32, kind="ExternalInput")
    bk = nc.dram_tensor("bk", (DG,), f32, kind="ExternalInput")
    o_t = nc.dram_tensor("o_t", (D, S), f32, kind="ExternalOutput")

    with tile.TileContext(nc) as tc:
        with (
            tc.tile_pool(name="persist", bufs=1) as persist,
            tc.tile_pool(name="wp", bufs=3) as wp,
            tc.tile_pool(name="xp", bufs=8) as xp,
            tc.tile_pool(name="xvp", bufs=8) as xvp,
            tc.tile_pool(name="ptp", bufs=30) as ptp,
            tc.tile_pool(name="pvs", bufs=2) as pvsp,
            tc.tile_pool(name="rbp", bufs=2) as rbp,
            tc.tile_pool(name="osb", bufs=2) as osbp,
            tc.tile_pool(name="sps", bufs=3, space="PSUM") as sps,
            tc.tile_pool(name="mps", bufs=2, space="PSUM") as mps,
        ):
            # ---- persistent tensors -------------------------------------
            QT = [persist.tile([128, S], QK_DT, tag=f"qt{p}", name=f"qt{p}")
                  for p in range(PB)]
            KT = [persist.tile([128, S], QK_DT, tag=f"kt{p}", name=f"kt{p}")
                  for p in range(PB)]
            VA = [persist.tile([128, H_LOC, DK + 1], bf16, tag=f"va{j}",
                               name=f"va{j}") for j in range(JB)]
            AT = [persist.tile([128, S], bf16, tag=f"at{p}", name=f"at{p}")
                  for p in range(PB)]

            bq_t = persist.tile([128, PB], f32, tag="bq")
            bk_t = persist.tile([128, PB], f32, tag="bk")
            nc.sync.dma_start(out=bq_t, in_=bq.rearrange("(pb p) -> p pb", p=128))
            nc.sync.dma_start(out=bk_t, in_=bk.rearrange("(pb p) -> p pb", p=128))
            for j in range(JB):
                nc.vector.memset(VA[j][:, :, DK:DK + 1], 1.0)

            # ---- V projection (bf16), emitted after the first scores
            # block so it becomes PE filler inside the ACT-bound window
            def v_proj():
                wv_t = wp.tile([128, KB, N], bf16, tag="w", name="wv_t")
                nc.sync.dma_start(
                    out=wv_t, in_=wv.rearrange("(kb p) n -> p kb n", p=128)
                )
                for jg in range(4):
                    xc = []
                    for kb in range(KB):
                        t = xvp.tile([128, N], bf16, tag="xcv", name="xcv")
                        nc.sync.dma_start(
                            out=t,
                            in_=xv[kb * 128:(kb + 1) * 128,
                                   jg * N:(jg + 1) * N],
                        )
                        xc.append(t)
                    for jj in range(4):
                        j = jg * 4 + jj
                        ps = mps.tile([128, N], f32, tag="mm", name="vps")
                        for kb in range(KB):
                            nc.tensor.matmul(
                                ps,
                                xc[kb][:, jj * 128:(jj + 1) * 128],
                                wv_t[:, kb, :],
                                start=(kb == 0),
                                stop=(kb == KB - 1),
                            )
                        nc.vector.tensor_copy(
                            VA[j][:, :, 0:DK],
                            ps.rearrange("p (h e) -> p h e", e=DK),
                        )

            # ---- K then Q projections (feature-major output) ------------
            def project_qk(x_dram, w_dram, bias_t, out_tiles, label):
                w_t = wp.tile([128, KB, N], QK_DT, tag="w", name=f"w_{label}")
                nc.sync.dma_start(
                    out=w_t, in_=w_dram.rearrange("(kb p) n -> p kb n", p=128)
                )
                for ic in range(IC):
                    xc = []
                    for kb in range(KB):
                        t = xp.tile([128, N], QK_DT, tag="xc", name=f"xc_{label}")
                        nc.sync.dma_start(
                            out=t,
                            in_=x_dram[kb * 128:(kb + 1) * 128,
                                       ic * N:(ic + 1) * N],
                        )
                        xc.append(t)
                    for pb in range(PB):
                        ps = mps.tile([128, N], f32, tag="mm", name=f"ps_{label}")
                        for kb in range(KB):
                            nc.tensor.matmul(
                                ps,
                                w_t[:, kb, pb * 128:(pb + 1) * 128],
                                xc[kb],
                                start=(kb == 0),
                                stop=(kb == KB - 1),
                            )
                        nc.vector.tensor_scalar_add(
                            out_tiles[pb][:, ic * N:(ic + 1) * N],
                            ps,
                            bias_t[:, pb:pb + 1],
                        )

            project_qk(xk, wk, bk_t, KT, "k")

            # ---- attention, software-pipelined ---------------------------
            # Groups run ic-major.  Emission order per group: scores+exp
            # (high priority - keeps ScalarE fed), then the PREVIOUS
            # group's PV + normalization as PE filler inside the
            # ACT-bound scores window.  Q projection for each i-chunk and
            # the previous i-chunk's output projection are emitted as
            # filler too.
            wq_t = wp.tile([128, KB, N], QK_DT, tag="w", name="w_q")
            nc.sync.dma_start(
                out=wq_t, in_=wq.rearrange("(kb p) n -> p kb n", p=128)
            )
            wo_t = wp.tile([128, PB, D], bf16, tag="w", name="wo_t")
            nc.sync.dma_start(
                out=wo_t, in_=wo.rearrange("(pb p) n -> p pb n", p=128)
            )

            def q_proj_ic(ic):
                xc = []
                for kb in range(KB):
                    t = xp.tile([128, N], QK_DT, tag="xc", name="xc_q")
                    nc.sync.dma_start(
                        out=t,
                        in_=xq[kb * 128:(kb + 1) * 128, ic * N:(ic + 1) * N],
                    )
                    xc.append(t)
                for pb in range(PB):
                    ps = mps.tile([128, N], f32, tag="mm", name="ps_q")
                    for kb in range(KB):
                        nc.tensor.matmul(
                            ps,
                            wq_t[:, kb, pb * 128:(pb + 1) * 128],
                            xc[kb],
                            start=(kb == 0),
                            stop=(kb == KB - 1),
                        )
                    nc.vector.tensor_scalar_add(
                        QT[pb][:, ic * N:(ic + 1) * N],
                        ps,
                        bq_t[:, pb:pb + 1],
                    )

            def scores_phase(pair, ic):
                pts = []
                for j in range(JB):
                    s_ps = sps.tile([128, 2 * N], f32, tag="s", name="s_ps")
                    nc.tensor.matmul(
                        s_ps[:, 0:N],
                        KT[pair][0:64, j * 128:(j + 1) * 128],
                        QT[pair][0:64, ic * N:(ic + 1) * N],
                        start=True, stop=True,
                    )
                    nc.tensor.matmul(
                        s_ps[:, N:2 * N],
                        KT[pair][64:128, j * 128:(j + 1) * 128],
                        QT[pair][64:128, ic * N:(ic + 1) * N],
                        start=True, stop=True,
                        tile_position=(64, 0),
                    )
                    pt = ptp.tile([128, 2 * N], bf16, tag="pt", name="pt")
                    nc.scalar.activation(pt, s_ps, Exp, scale=0.125)
                    pts.append(pt)
                return pts

            def pv_phase(pair, ic, pts):
                pv = [
                    mps.tile([DK + 1, N], f32, tag="mm", name="pv0"),
                    mps.tile([DK + 1, N], f32, tag="mm", name="pv1"),
                ]
                for h2 in range(2):
                    for j in range(JB):
                        nc.tensor.matmul(
                            pv[h2],
                            VA[j][:, 2 * pair + h2, :],
                            pts[j][:, h2 * N:(h2 + 1) * N],
                            start=(j == 0),
                            stop=(j == JB - 1),
                        )
                for h2 in range(2):
                    pvs = pvsp.tile([DK + 1, N], f32, tag="pvs", name="pvs")
                    nc.vector.tensor_copy(pvs[0:DK, :], pv[h2][0:DK, :])
                    den = rbp.tile([1, N], f32, tag="den", name="den")
                    nc.vector.tensor_copy(den, pv[h2][DK:DK + 1, :])
                    rbr = rbp.tile([64, N], f32, tag="rbr", name="rbr")
                    nc.gpsimd.partition_broadcast(rbr, den)
                    rb = rbp.tile([64, N], f32, tag="rb", name="rb")
                    nc.vector.reciprocal_approx_fast(rb, rbr)
                    dst = AT[pair][h2 * 64:(h2 + 1) * 64, ic * N:(ic + 1) * N]
                    nc.vector.tensor_tensor(
                        out=dst, in0=pvs[0:DK, :], in1=rb, op=MULT
                    )

            def oproj_ic(ic):
                for dob in range(KB):
                    ops = mps.tile([128, N], f32, tag="mm", name="ops")
                    for pb in range(PB):
                        nc.tensor.matmul(
                            ops,
                            wo_t[:, pb, dob * 128:(dob + 1) * 128],
                            AT[pb][:, ic * N:(ic + 1) * N],
                            start=(pb == 0),
                            stop=(pb == PB - 1),
                        )
                    ob = osbp.tile([128, N], f32, tag="ob", name="ob")
                    nc.vector.tensor_copy(ob, ops)
                    nc.sync.dma_start(
                        out=o_t[dob * 128:(dob + 1) * 128, ic * N:(ic + 1) * N],
                        in_=ob,
                    )

            prev = None          # (pair, ic, pts) of the unconsumed group
            for ic in range(IC):
                q_proj_ic(ic)
                for pair in range(PB):
                    pts = scores_phase(pair, ic)
                    if ic == 0 and pair == 0:
                        v_proj()
                    if prev is not None:
                        pv_phase(*prev)
                        if pair == 1 and ic > 0:
                            oproj_ic(ic - 1)
                    prev = (pair, ic, pts)
            pv_phase(*prev)
            oproj_ic(IC - 1)

    nc.compile()
    return nc


_NC_CACHE = None


def _get_nc():
    global _NC_CACHE
    if _NC_CACHE is None:
        _NC_CACHE = _build()
    return _NC_CACHE


def kernel(q, k, v, W_q, b_q, W_k, b_k, W_v, b_v, W_o, b_o):
    import ml_dtypes

    q = np.asarray(q, dtype=np.float32)
    k = np.asarray(k, dtype=np.float32)
    v = np.asarray(v, dtype=np.float32)
    W_q = np.asarray(W_q, dtype=np.float32)
    W_k = np.asarray(W_k, dtype=np.float32)
    W_v = np.asarray(W_v, dtype=np.float32)
    W_o = np.asarray(W_o, dtype=np.float32)
    b_q = np.asarray(b_q, dtype=np.float32)
    b_k = np.asarray(b_k, dtype=np.float32)
    b_v = np.asarray(b_v, dtype=np.float32)
    b_o = np.asarray(b_o, dtype=np.float32)

    bf = ml_dtypes.bfloat16
    xq_t = [np.ascontiguousarray(q[b].T).astype(bf) for b in range(B)]
    xk_t = [np.ascontiguousarray(k[b].T).astype(bf) for b in range(B)]
    xv_t = [np.ascontiguousarray(v[b].T).astype(bf) for b in range(B)]
    wq_s = [np.ascontiguousarray(W_q[:, g * DG:(g + 1) * DG]).astype(bf)
            for g in range(2)]
    wk_s = [np.ascontiguousarray(W_k[:, g * DG:(g + 1) * DG]).astype(bf)
            for g in range(2)]
    wv_s = [np.ascontiguousarray(W_v[:, g * DG:(g + 1) * DG]).astype(bf)
            for g in range(2)]
    wo_s = [np.ascontiguousarray(W_o[g * DG:(g + 1) * DG, :]).astype(bf)
            for g in range(2)]

    in_maps = []
    for c in range(8):
        b, g = c // 2, c % 2
        in_maps.append({
            "xq": xq_t[b], "xk": xk_t[b], "xv": xv_t[b],
            "wq": wq_s[g], "wk": wk_s[g], "wv": wv_s[g], "wo": wo_s[g],
            "bq": b_q[g * DG:(g + 1) * DG],
            "bk": b_k[g * DG:(g + 1) * DG],
        })

    nc = _get_nc()
    trace = bool(int(os.environ.get("KERNEL_TRACE", "0")))
    if trace:
        try:
            import axon_profile_shim
            axon_profile_shim.install()
        except Exception:
            pass
    res = run_bass_kernel_spmd(nc, in_maps, core_ids=list(range(8)), trace=trace)
    if res.exec_time_ns is not None:
        print(f"HW exec time: {res.exec_time_ns} ns", flush=True)

    out = np.empty((B, S, D), dtype=np.float32)
    # b_v is an exact constant output offset: softmax rows sum to 1, so
    # attn @ (V + 1 b_v^T) @ W_o = attn @ V @ W_o + b_v @ W_o.
    bv_off = [b_v[g * DG:(g + 1) * DG] @ W_o[g * DG:(g + 1) * DG, :]
              for g in range(2)]
    full_bias = b_o + bv_off[0] + bv_off[1]
    for b in range(B):
        part = res.results[2 * b]["o_t"] + res.results[2 * b + 1]["o_t"]
        out[b] = part.T + full_bias
    return out


# revision 24
# speedup vs baseline: 1.2800x; 1.0527x over previous
"""Multi-head attention (B=4, S=2048, D=1024, H=16) on 8 TRN2 NeuronCores.

Sharding (data + head parallel): core c handles batch b = c//2 and head
group g = c%2 (8 of the 16 heads, feature columns 512g:512(g+1)).
Each core computes its heads' full attention locally and a partial
output projection; the host sums the two partials per batch and adds
b_o plus the b_v @ W_o term (softmax rows sum to 1, so the V bias is an
exact constant output offset and never touches the device).

On-device layout is feature-major ("transposed"): activations live as
[feature, seq] so every linear layer is matmul(lhsT=W-block, rhs=x^T)
with W loaded from HBM exactly as stored (in, out).  The host passes
q/k/v pre-transposed per batch and receives the partial output
transposed back.

Pipeline per core (all matmul moving dims 512, bf16 compute with fp32
PSUM accumulation; measured absmax relative error vs the fp32
reference ~5.4e-3):
  V     = x @ Wv (bf16), natural [seq, feat] layout, evacuated with a
          ones column per head (V_aug [j, 8*65])
  KT/QT = (x @ Wk/Wq)^T (bf16) + bias (per-partition) on evacuation
  scores^T[j, i] per head pair via row-packed K=64 matmuls (the two
          heads run concurrently on separate 64-row tile groups),
          softmax exp on ScalarE directly from PSUM ([128,1024] grain,
          scale=1/8 folded in; no max subtraction: scores ~ N(0,1) so
          exp is safely bounded), probabilities written bf16
  PV    = V_aug^T @ P^T accumulated over 16 j-blocks in PSUM -> rows
          0:64 head output (transposed), row 64 softmax denominator.
          PSUM rows are evacuated to SBUF immediately (frees the bank);
          normalization = gpsimd partition_broadcast of the denominator
          + reciprocal_approx_fast + vector multiply, off the critical
          path, bf16 attnT out.
  out   = Wo^T @ attnT (bf16), fp32 partial written to HBM.
"""

import os

import numpy as np

import concourse.bass as bass  # noqa: F401
import concourse.mybir as mybir
import concourse.tile as tile
from concourse import bacc
from concourse.bass_utils import run_bass_kernel_spmd

f32 = mybir.dt.float32
bf16 = mybir.dt.bfloat16
Exp = mybir.ActivationFunctionType.Exp
MULT = mybir.AluOpType.mult

B, S, D = 4, 2048, 1024
H_LOC = 8
DK = 64
DG = 512
KB = D // 128
PB = DG // 128
JB = S // 128
IC = S // 512
N = 512
QK_DT = bf16


def _build():
    nc = bacc.Bacc("TRN2")

    xq = nc.dram_tensor("xq", (D, S), QK_DT, kind="ExternalInput")
    xk = nc.dram_tensor("xk", (D, S), QK_DT, kind="ExternalInput")
    xv = nc.dram_tensor("xv", (D, S), bf16, kind="ExternalInput")
    wq = nc.dram_tensor("wq", (D, DG), QK_DT, kind="ExternalInput")
    wk = nc.dram_tensor("wk", (D, DG), QK_DT, kind="ExternalInput")
    wv = nc.dram_tensor("wv", (D, DG), bf16, kind="ExternalInput")
    wo = nc.dram_tensor("wo", (DG, D), bf16, kind="ExternalInput")
    bq = nc.dram_tensor("bq", (DG,), f32, kind="ExternalInput")
    bk = nc.dram_tensor("bk", (DG,), f32, kind="ExternalInput")
    o_t = nc.dram_tensor("o_t", (D, S), f32, kind="ExternalOutput")

    with tile.TileContext(nc) as tc:
        with (
            tc.tile_pool(name="persist", bufs=1) as persist,
            tc.tile_pool(name="wp", bufs=3) as wp,
            tc.tile_pool(name="xp", bufs=12) as xp,
            tc.tile_pool(name="xvp", bufs=8) as xvp,
            tc.tile_pool(name="ptp", bufs=30) as ptp,
            tc.tile_pool(name="pvs", bufs=2) as pvsp,
            tc.tile_pool(name="rbp", bufs=2) as rbp,
            tc.tile_pool(name="osb", bufs=2) as osbp,
            tc.tile_pool(name="sps", bufs=3, space="PSUM") as sps,
            tc.tile_pool(name="mps", bufs=2, space="PSUM") as mps,
        ):
            # ---- persistent tensors -------------------------------------
            QT = [persist.tile([128, S], QK_DT, tag=f"qt{p}", name=f"qt{p}")
                  for p in range(PB)]
            KT = [persist.tile([128, S], QK_DT, tag=f"kt{p}", name=f"kt{p}")
                  for p in range(PB)]
            VA = [persist.tile([128, H_LOC, DK + 1], bf16, tag=f"va{j}",
                               name=f"va{j}") for j in range(JB)]
            AT = [persist.tile([128, S], bf16, tag=f"at{p}", name=f"at{p}")
                  for p in range(PB)]

            bq_t = persist.tile([128, PB], f32, tag="bq")
            bk_t = persist.tile([128, PB], f32, tag="bk")
            nc.sync.dma_start(out=bq_t, in_=bq.rearrange("(pb p) -> p pb", p=128))
            nc.sync.dma_start(out=bk_t, in_=bk.rearrange("(pb p) -> p pb", p=128))
            for j in range(JB):
                nc.vector.memset(VA[j][:, :, DK:DK + 1], 1.0)

            # ---- V projection (bf16), emitted after the first scores
            # block so it becomes PE filler inside the ACT-bound window
            def v_proj():
                wv_t = wp.tile([128, KB, N], bf16, tag="w", name="wv_t")
                nc.sync.dma_start(
                    out=wv_t, in_=wv.rearrange("(kb p) n -> p kb n", p=128)
                )
                for jg in range(4):
                    xc = []
                    for kb in range(KB):
                        t = xvp.tile([128, N], bf16, tag="xcv", name="xcv")
                        nc.sync.dma_start(
                            out=t,
                            in_=xv[kb * 128:(kb + 1) * 128,
                                   jg * N:(jg + 1) * N],
                        )
                        xc.append(t)
                    for jj in range(4):
                        j = jg * 4 + jj
                        ps = mps.tile([128, N], f32, tag="mm", name="vps")
                        for kb in range(KB):
                            nc.tensor.matmul(
                                ps,
                                xc[kb][:, jj * 128:(jj + 1) * 128],
                                wv_t[:, kb, :],
                                start=(kb == 0),
                                stop=(kb == KB - 1),
                            )
                        nc.vector.tensor_copy(
                            VA[j][:, :, 0:DK],
                            ps.rearrange("p (h e) -> p h e", e=DK),
                        )

            # ---- K then Q projections (feature-major output) ------------
            def project_qk(x_dram, w_dram, bias_t, out_tiles, label):
                w_t = wp.tile([128, KB, N], QK_DT, tag="w", name=f"w_{label}")
                nc.sync.dma_start(
                    out=w_t, in_=w_dram.rearrange("(kb p) n -> p kb n", p=128)
                )
                for ic in range(IC):
                    xc = []
                    for kb in range(KB):
                        t = xp.tile([128, N], QK_DT, tag="xc", name=f"xc_{label}")
                        nc.sync.dma_start(
                            out=t,
                            in_=x_dram[kb * 128:(kb + 1) * 128,
                                       ic * N:(ic + 1) * N],
                        )
                        xc.append(t)
                    for pb in range(PB):
                        ps = mps.tile([128, N], f32, tag="mm", name=f"ps_{label}")
                        for kb in range(KB):
                            nc.tensor.matmul(
                                ps,
                                w_t[:, kb, pb * 128:(pb + 1) * 128],
                                xc[kb],
                                start=(kb == 0),
                                stop=(kb == KB - 1),
                            )
                        nc.vector.tensor_scalar_add(
                            out_tiles[pb][:, ic * N:(ic + 1) * N],
                            ps,
                            bias_t[:, pb:pb + 1],
                        )

            project_qk(xk, wk, bk_t, KT, "k")

            # ---- attention, software-pipelined ---------------------------
            # Groups run ic-major.  Emission order per group: scores+exp
            # (high priority - keeps ScalarE fed), then the PREVIOUS
            # group's PV + normalization as PE filler inside the
            # ACT-bound scores window.  Q projection for each i-chunk and
            # the previous i-chunk's output projection are emitted as
            # filler too.
            wq_t = wp.tile([128, KB, N], QK_DT, tag="w", name="w_q")
            nc.sync.dma_start(
                out=wq_t, in_=wq.rearrange("(kb p) n -> p kb n", p=128)
            )
            wo_t = wp.tile([128, PB, D], bf16, tag="w", name="wo_t")
            nc.sync.dma_start(
                out=wo_t, in_=wo.rearrange("(pb p) n -> p pb n", p=128)
            )

            def q_proj_ic(ic):
                xc = []
                for kb in range(KB):
                    t = xp.tile([128, N], QK_DT, tag="xc", name="xc_q")
                    nc.sync.dma_start(
                        out=t,
                        in_=xq[kb * 128:(kb + 1) * 128, ic * N:(ic + 1) * N],
                    )
                    xc.append(t)
                for pb in range(PB):
                    ps = mps.tile([128, N], f32, tag="mm", name="ps_q")
                    for kb in range(KB):
                        nc.tensor.matmul(
                            ps,
                            wq_t[:, kb, pb * 128:(pb + 1) * 128],
                            xc[kb],
                            start=(kb == 0),
                            stop=(kb == KB - 1),
                        )
                    nc.vector.tensor_scalar_add(
                        QT[pb][:, ic * N:(ic + 1) * N],
                        ps,
                        bq_t[:, pb:pb + 1],
                    )

            def scores_phase(pair, ic):
                pts = []
                for j in range(JB):
                    s_ps = sps.tile([128, 2 * N], f32, tag="s", name="s_ps")
                    nc.tensor.matmul(
                        s_ps[:, 0:N],
                        KT[pair][0:64, j * 128:(j + 1) * 128],
                        QT[pair][0:64, ic * N:(ic + 1) * N],
                        start=True, stop=True,
                    )
                    nc.tensor.matmul(
                        s_ps[:, N:2 * N],
                        KT[pair][64:128, j * 128:(j + 1) * 128],
                        QT[pair][64:128, ic * N:(ic + 1) * N],
                        start=True, stop=True,
                        tile_position=(64, 0),
                    )
                    pt = ptp.tile([128, 2 * N], bf16, tag="pt", name="pt")
                    nc.scalar.activation(pt, s_ps, Exp, scale=0.125)
                    pts.append(pt)
                return pts

            def pv_phase(pair, ic, pts):
                pv = [
                    mps.tile([DK + 1, N], f32, tag="mm", name="pv0"),
                    mps.tile([DK + 1, N], f32, tag="mm", name="pv1"),
                ]
                for h2 in range(2):
                    for j in range(JB):
                        nc.tensor.matmul(
                            pv[h2],
                            VA[j][:, 2 * pair + h2, :],
                            pts[j][:, h2 * N:(h2 + 1) * N],
                            start=(j == 0),
                            stop=(j == JB - 1),
                        )
                for h2 in range(2):
                    pvs = pvsp.tile([DK + 1, N], f32, tag="pvs", name="pvs")
                    nc.vector.tensor_copy(pvs[0:DK, :], pv[h2][0:DK, :])
                    den = rbp.tile([1, N], f32, tag="den", name="den")
                    nc.vector.tensor_copy(den, pv[h2][DK:DK + 1, :])
                    rbr = rbp.tile([64, N], f32, tag="rbr", name="rbr")
                    nc.gpsimd.partition_broadcast(rbr, den)
                    rb = rbp.tile([64, N], f32, tag="rb", name="rb")
                    nc.vector.reciprocal_approx_fast(rb, rbr)
                    dst = AT[pair][h2 * 64:(h2 + 1) * 64, ic * N:(ic + 1) * N]
                    nc.vector.tensor_tensor(
                        out=dst, in0=pvs[0:DK, :], in1=rb, op=MULT
                    )

            def oproj_ic(ic):
                for dob in range(KB):
                    ops = mps.tile([128, N], f32, tag="mm", name="ops")
                    for pb in range(PB):
                        nc.tensor.matmul(
                            ops,
                            wo_t[:, pb, dob * 128:(dob + 1) * 128],
                            AT[pb][:, ic * N:(ic + 1) * N],
                            start=(pb == 0),
                            stop=(pb == PB - 1),
                        )
                    ob = osbp.tile([128, N], f32, tag="ob", name="ob")
                    nc.vector.tensor_copy(ob, ops)
                    nc.sync.dma_start(
                        out=o_t[dob * 128:(dob + 1) * 128, ic * N:(ic + 1) * N],
                        in_=ob,
                    )

            prev = None          # (pair, ic, pts) of the unconsumed group
            for ic in range(IC):
                q_proj_ic(ic)
                for pair in range(PB):
                    pts = scores_phase(pair, ic)
                    if ic == 0 and pair == 0:
                        v_proj()
                    if prev is not None:
                        pv_phase(*prev)
                        if pair == 1 and ic > 0:
                            oproj_ic(ic - 1)
                    prev = (pair, ic, pts)
            pv_phase(*prev)
            oproj_ic(IC - 1)

    nc.compile()
    return nc


_NC_CACHE = None


def _get_nc():
    global _NC_CACHE
    if _NC_CACHE is None:
        _NC_CACHE = _build()
    return _NC_CACHE


def kernel(q, k, v, W_q, b_q, W_k, b_k, W_v, b_v, W_o, b_o):
    import ml_dtypes

    q = np.asarray(q, dtype=np.float32)
    k = np.asarray(k, dtype=np.float32)
    v = np.asarray(v, dtype=np.float32)
    W_q = np.asarray(W_q, dtype=np.float32)
    W_k = np.asarray(W_k, dtype=np.float32)
    W_v = np.asarray(W_v, dtype=np.float32)
    W_o = np.asarray(W_o, dtype=np.float32)
    b_q = np.asarray(b_q, dtype=np.float32)
    b_k = np.asarray(b_k, dtype=np.float32)
    b_v = np.asarray(b_v, dtype=np.float32)
    b_o = np.asarray(b_o, dtype=np.float32)

    bf = ml_dtypes.bfloat16
    xq_t = [np.ascontiguousarray(q[b].T).astype(bf) for b in range(B)]
    xk_t = [np.ascontiguousarray(k[b].T).astype(bf) for b in range(B)]
    xv_t = [np.ascontiguousarray(v[b].T).astype(bf) for b in range(B)]
    wq_s = [np.ascontiguousarray(W_q[:, g * DG:(g + 1) * DG]).astype(bf)
            for g in range(2)]
    wk_s = [np.ascontiguousarray(W_k[:, g * DG:(g + 1) * DG]).astype(bf)
            for g in range(2)]
    wv_s = [np.ascontiguousarray(W_v[:, g * DG:(g + 1) * DG]).astype(bf)
            for g in range(2)]
    wo_s = [np.ascontiguousarray(W_o[g * DG:(g + 1) * DG, :]).astype(bf)
            for g in range(2)]

    in_maps = []
    for c in range(8):
        b, g = c // 2, c % 2
        in_maps.append({
            "xq": xq_t[b], "xk": xk_t[b], "xv": xv_t[b],
            "wq": wq_s[g], "wk": wk_s[g], "wv": wv_s[g], "wo": wo_s[g],
            "bq": b_q[g * DG:(g + 1) * DG],
            "bk": b_k[g * DG:(g + 1) * DG],
        })

    nc = _get_nc()
    trace = bool(int(os.environ.get("KERNEL_TRACE", "0")))
    if trace:
        try:
            import axon_profile_shim
            axon_profile_shim.install()
        except Exception:
            pass
    res = run_bass_kernel_spmd(nc, in_maps, core_ids=list(range(8)), trace=trace)
    if res.exec_time_ns is not None:
        print(f"HW exec time: {res.exec_time_ns} ns", flush=True)

    out = np.empty((B, S, D), dtype=np.float32)
    # b_v is an exact constant output offset: softmax rows sum to 1, so
    # attn @ (V + 1 b_v^T) @ W_o = attn @ V @ W_o + b_v @ W_o.
    bv_off = [b_v[g * DG:(g + 1) * DG] @ W_o[g * DG:(g + 1) * DG, :]
              for g in range(2)]
    full_bias = b_o + bv_off[0] + bv_off[1]
    for b in range(B):
        part = res.results[2 * b]["o_t"] + res.results[2 * b + 1]["o_t"]
        out[b] = part.T + full_bias
    return out


# revision 27
# speedup vs baseline: 1.2805x; 1.0004x over previous
"""Multi-head attention (B=4, S=2048, D=1024, H=16) on 8 TRN2 NeuronCores.

Sharding (data + head parallel): core c handles batch b = c//2 and head
group g = c%2 (8 of the 16 heads, feature columns 512g:512(g+1)).
Each core computes its heads' full attention locally and a partial
output projection; the host sums the two partials per batch and adds
b_o plus the b_v @ W_o term (softmax rows sum to 1, so the V bias is an
exact constant output offset and never touches the device).

On-device layout is feature-major ("transposed"): activations live as
[feature, seq] so every linear layer is matmul(lhsT=W-block, rhs=x^T)
with W loaded from HBM exactly as stored (in, out).  The host passes
q/k/v pre-transposed per batch and receives the partial output
transposed back.

Pipeline per core (all matmul moving dims 512, bf16 compute with fp32
PSUM accumulation; measured absmax relative error vs the fp32
reference ~5.4e-3):
  V     = x @ Wv (bf16), natural [seq, feat] layout, evacuated with a
          ones column per head (V_aug [j, 8*65])
  KT/QT = (x @ Wk/Wq)^T (bf16) + bias (per-partition) on evacuation
  scores^T[j, i] per head pair via row-packed K=64 matmuls (the two
          heads run concurrently on separate 64-row tile groups),
          softmax exp on ScalarE directly from PSUM ([128,1024] grain,
          scale=1/8 folded in; no max subtraction: scores ~ N(0,1) so
          exp is safely bounded), probabilities written bf16
  PV    = V_aug^T @ P^T accumulated over 16 j-blocks in PSUM -> rows
          0:64 head output (transposed), row 64 softmax denominator.
          PSUM rows are evacuated to SBUF immediately (frees the bank);
          normalization = gpsimd partition_broadcast of the denominator
          + reciprocal_approx_fast + vector multiply, off the critical
          path, bf16 attnT out.
  out   = Wo^T @ attnT (bf16), fp32 partial written to HBM.
"""

import os

import numpy as np

import concourse.bass as bass  # noqa: F401
import concourse.mybir as mybir
import concourse.tile as tile
from concourse import bacc
from concourse.bass_utils import run_bass_kernel_spmd

f32 = mybir.dt.float32
bf16 = mybir.dt.bfloat16
Exp = mybir.ActivationFunctionType.Exp
MULT = mybir.AluOpType.mult

B, S, D = 4, 2048, 1024
H_LOC = 8
DK = 64
DG = 512
KB = D // 128
PB = DG // 128
JB = S // 128
IC = S // 512
N = 512
QK_DT = bf16


def _build():
    nc = bacc.Bacc("TRN2")

    xq = nc.dram_tensor("xq", (D, S), QK_DT, kind="ExternalInput")
    xk = nc.dram_tensor("xk", (D, S), QK_DT, kind="ExternalInput")
    xv = nc.dram_tensor("xv", (D, S), bf16, kind="ExternalInput")
    wq = nc.dram_tensor("wq", (D, DG), QK_DT, kind="ExternalInput")
    wk = nc.dram_tensor("wk", (D, DG), QK_DT, kind="ExternalInput")
    wv = nc.dram_tensor("wv", (D, DG), bf16, kind="ExternalInput")
    wo = nc.dram_tensor("wo", (DG, D), bf16, kind="ExternalInput")
    bq = nc.dram_tensor("bq", (DG,), f32, kind="ExternalInput")
    bk = nc.dram_tensor("bk", (DG,), f32, kind="ExternalInput")
    o_t = nc.dram_tensor("o_t", (D, S), f32, kind="ExternalOutput")

    with tile.TileContext(nc) as tc:
        with (
            tc.tile_pool(name="persist", bufs=1) as persist,
            tc.tile_pool(name="wp", bufs=3) as wp,
            tc.tile_pool(name="xp", bufs=12) as xp,
            tc.tile_pool(name="xvp", bufs=12) as xvp,
            tc.tile_pool(name="ptp", bufs=30) as ptp,
            tc.tile_pool(name="pvs", bufs=2) as pvsp,
            tc.tile_pool(name="rbp", bufs=2) as rbp,
            tc.tile_pool(name="osb", bufs=2) as osbp,
            tc.tile_pool(name="sps", bufs=3, space="PSUM") as sps,
            tc.tile_pool(name="mps", bufs=2, space="PSUM") as mps,
        ):
            # ---- persistent tensors -------------------------------------
            QT = [persist.tile([128, S], QK_DT, tag=f"qt{p}", name=f"qt{p}")
                  for p in range(PB)]
            KT = [persist.tile([128, S], QK_DT, tag=f"kt{p}", name=f"kt{p}")
                  for p in range(PB)]
            VA = [persist.tile([128, H_LOC, DK + 1], bf16, tag=f"va{j}",
                               name=f"va{j}") for j in range(JB)]
            AT = [persist.tile([128, S], bf16, tag=f"at{p}", name=f"at{p}")
                  for p in range(PB)]

            bq_t = persist.tile([128, PB], f32, tag="bq")
            bk_t = persist.tile([128, PB], f32, tag="bk")
            nc.sync.dma_start(out=bq_t, in_=bq.rearrange("(pb p) -> p pb", p=128))
            nc.sync.dma_start(out=bk_t, in_=bk.rearrange("(pb p) -> p pb", p=128))
            for j in range(JB):
                nc.vector.memset(VA[j][:, :, DK:DK + 1], 1.0)

            # ---- V projection (bf16), emitted after the first scores
            # block so it becomes PE filler inside the ACT-bound window
            def v_proj():
                wv_t = wp.tile([128, KB, N], bf16, tag="w", name="wv_t")
                nc.sync.dma_start(
                    out=wv_t, in_=wv.rearrange("(kb p) n -> p kb n", p=128)
                )
                for jg in range(4):
                    xc = []
                    for kb in range(KB):
                        t = xvp.tile([128, N], bf16, tag="xcv", name="xcv")
                        nc.sync.dma_start(
                            out=t,
                            in_=xv[kb * 128:(kb + 1) * 128,
                                   jg * N:(jg + 1) * N],
                        )
                        xc.append(t)
                    for jj in range(4):
                        j = jg * 4 + jj
                        ps = mps.tile([128, N], f32, tag="mm", name="vps")
                        for kb in range(KB):
                            nc.tensor.matmul(
                                ps,
                                xc[kb][:, jj * 128:(jj + 1) * 128],
                                wv_t[:, kb, :],
                                start=(kb == 0),
                                stop=(kb == KB - 1),
                            )
                        nc.vector.tensor_copy(
                            VA[j][:, :, 0:DK],
                            ps.rearrange("p (h e) -> p h e", e=DK),
                        )

            # ---- K then Q projections (feature-major output) ------------
            def project_qk(x_dram, w_dram, bias_t, out_tiles, label):
                w_t = wp.tile([128, KB, N], QK_DT, tag="w", name=f"w_{label}")
                nc.sync.dma_start(
                    out=w_t, in_=w_dram.rearrange("(kb p) n -> p kb n", p=128)
                )
                for ic in range(IC):
                    xc = []
                    for kb in range(KB):
                        t = xp.tile([128, N], QK_DT, tag="xc", name=f"xc_{label}")
                        nc.sync.dma_start(
                            out=t,
                            in_=x_dram[kb * 128:(kb + 1) * 128,
                                       ic * N:(ic + 1) * N],
                        )
                        xc.append(t)
                    for pb in range(PB):
                        ps = mps.tile([128, N], f32, tag="mm", name=f"ps_{label}")
                        for kb in range(KB):
                            nc.tensor.matmul(
                                ps,
                                w_t[:, kb, pb * 128:(pb + 1) * 128],
                                xc[kb],
                                start=(kb == 0),
                                stop=(kb == KB - 1),
                            )
                        nc.vector.tensor_scalar_add(
                            out_tiles[pb][:, ic * N:(ic + 1) * N],
                            ps,
                            bias_t[:, pb:pb + 1],
                        )

            project_qk(xk, wk, bk_t, KT, "k")

            # ---- attention, software-pipelined ---------------------------
            # Groups run ic-major.  Emission order per group: scores+exp
            # (high priority - keeps ScalarE fed), then the PREVIOUS
            # group's PV + normalization as PE filler inside the
            # ACT-bound scores window.  Q projection for each i-chunk and
            # the previous i-chunk's output projection are emitted as
            # filler too.
            wq_t = wp.tile([128, KB, N], QK_DT, tag="w", name="w_q")
            nc.sync.dma_start(
                out=wq_t, in_=wq.rearrange("(kb p) n -> p kb n", p=128)
            )
            wo_t = wp.tile([128, PB, D], bf16, tag="w", name="wo_t")
            nc.sync.dma_start(
                out=wo_t, in_=wo.rearrange("(pb p) n -> p pb n", p=128)
            )

            def q_proj_ic(ic):
                xc = []
                for kb in range(KB):
                    t = xp.tile([128, N], QK_DT, tag="xc", name="xc_q")
                    nc.sync.dma_start(
                        out=t,
                        in_=xq[kb * 128:(kb + 1) * 128, ic * N:(ic + 1) * N],
                    )
                    xc.append(t)
                for pb in range(PB):
                    ps = mps.tile([128, N], f32, tag="mm", name="ps_q")
                    for kb in range(KB):
                        nc.tensor.matmul(
                            ps,
                            wq_t[:, kb, pb * 128:(pb + 1) * 128],
                            xc[kb],
                            start=(kb == 0),
                            stop=(kb == KB - 1),
                        )
                    nc.vector.tensor_scalar_add(
                        QT[pb][:, ic * N:(ic + 1) * N],
                        ps,
                        bq_t[:, pb:pb + 1],
                    )

            def scores_phase(pair, ic):
                pts = []
                for j in range(JB):
                    s_ps = sps.tile([128, 2 * N], f32, tag="s", name="s_ps")
                    nc.tensor.matmul(
                        s_ps[:, 0:N],
                        KT[pair][0:64, j * 128:(j + 1) * 128],
                        QT[pair][0:64, ic * N:(ic + 1) * N],
                        start=True, stop=True,
                    )
                    nc.tensor.matmul(
                        s_ps[:, N:2 * N],
                        KT[pair][64:128, j * 128:(j + 1) * 128],
                        QT[pair][64:128, ic * N:(ic + 1) * N],
                        start=True, stop=True,
                        tile_position=(64, 0),
                    )
                    pt = ptp.tile([128, 2 * N], bf16, tag="pt", name="pt")
                    nc.scalar.activation(pt, s_ps, Exp, scale=0.125)
                    pts.append(pt)
                return pts

            def pv_phase(pair, ic, pts):
                pv = [
                    mps.tile([DK + 1, N], f32, tag="mm", name="pv0"),
                    mps.tile([DK + 1, N], f32, tag="mm", name="pv1"),
                ]
                for h2 in range(2):
                    for j in range(JB):
                        nc.tensor.matmul(
                            pv[h2],
                            VA[j][:, 2 * pair + h2, :],
                            pts[j][:, h2 * N:(h2 + 1) * N],
                            start=(j == 0),
                            stop=(j == JB - 1),
                        )
                for h2 in range(2):
                    pvs = pvsp.tile([DK + 1, N], f32, tag="pvs", name="pvs")
                    nc.vector.tensor_copy(pvs[0:DK, :], pv[h2][0:DK, :])
                    den = rbp.tile([1, N], f32, tag="den", name="den")
                    nc.vector.tensor_copy(den, pv[h2][DK:DK + 1, :])
                    rbr = rbp.tile([64, N], f32, tag="rbr", name="rbr")
                    nc.gpsimd.partition_broadcast(rbr, den)
                    rb = rbp.tile([64, N], f32, tag="rb", name="rb")
                    nc.vector.reciprocal_approx_fast(rb, rbr)
                    dst = AT[pair][h2 * 64:(h2 + 1) * 64, ic * N:(ic + 1) * N]
                    nc.vector.tensor_tensor(
                        out=dst, in0=pvs[0:DK, :], in1=rb, op=MULT
                    )

            def oproj_ic(ic):
                for dob in range(KB):
                    ops = mps.tile([128, N], f32, tag="mm", name="ops")
                    for pb in range(PB):
                        nc.tensor.matmul(
                            ops,
                            wo_t[:, pb, dob * 128:(dob + 1) * 128],
                            AT[pb][:, ic * N:(ic + 1) * N],
                            start=(pb == 0),
                            stop=(pb == PB - 1),
                        )
                    ob = osbp.tile([128, N], f32, tag="ob", name="ob")
                    nc.vector.tensor_copy(ob, ops)
                    nc.sync.dma_start(
                        out=o_t[dob * 128:(dob + 1) * 128, ic * N:(ic + 1) * N],
                        in_=ob,
                    )

            prev = None          # (pair, ic, pts) of the unconsumed group
            for ic in range(IC):
                q_proj_ic(ic)
                for pair in range(PB):
                    pts = scores_phase(pair, ic)
                    if ic == 0 and pair == 0:
                        v_proj()
                    if prev is not None:
                        pv_phase(*prev)
                        if pair == 1 and ic > 0:
                            oproj_ic(ic - 1)
                    prev = (pair, ic, pts)
            pv_phase(*prev)
            oproj_ic(IC - 1)

    nc.compile()
    return nc


_NC_CACHE = None


def _get_nc():
    global _NC_CACHE
    if _NC_CACHE is None:
        _NC_CACHE = _build()
    return _NC_CACHE


def kernel(q, k, v, W_q, b_q, W_k, b_k, W_v, b_v, W_o, b_o):
    import ml_dtypes

    q = np.asarray(q, dtype=np.float32)
    k = np.asarray(k, dtype=np.float32)
    v = np.asarray(v, dtype=np.float32)
    W_q = np.asarray(W_q, dtype=np.float32)
    W_k = np.asarray(W_k, dtype=np.float32)
    W_v = np.asarray(W_v, dtype=np.float32)
    W_o = np.asarray(W_o, dtype=np.float32)
    b_q = np.asarray(b_q, dtype=np.float32)
    b_k = np.asarray(b_k, dtype=np.float32)
    b_v = np.asarray(b_v, dtype=np.float32)
    b_o = np.asarray(b_o, dtype=np.float32)

    bf = ml_dtypes.bfloat16
    xq_t = [np.ascontiguousarray(q[b].T).astype(bf) for b in range(B)]
    xk_t = [np.ascontiguousarray(k[b].T).astype(bf) for b in range(B)]
    xv_t = [np.ascontiguousarray(v[b].T).astype(bf) for b in range(B)]
    wq_s = [np.ascontiguousarray(W_q[:, g * DG:(g + 1) * DG]).astype(bf)
            for g in range(2)]
    wk_s = [np.ascontiguousarray(W_k[:, g * DG:(g + 1) * DG]).astype(bf)
            for g in range(2)]
    wv_s = [np.ascontiguousarray(W_v[:, g * DG:(g + 1) * DG]).astype(bf)
            for g in range(2)]
    wo_s = [np.ascontiguousarray(W_o[g * DG:(g + 1) * DG, :]).astype(bf)
            for g in range(2)]

    in_maps = []
    for c in range(8):
        b, g = c // 2, c % 2
        in_maps.append({
            "xq": xq_t[b], "xk": xk_t[b], "xv": xv_t[b],
            "wq": wq_s[g], "wk": wk_s[g], "wv": wv_s[g], "wo": wo_s[g],
            "bq": b_q[g * DG:(g + 1) * DG],
            "bk": b_k[g * DG:(g + 1) * DG],
        })

    nc = _get_nc()
    trace = bool(int(os.environ.get("KERNEL_TRACE", "0")))
    if trace:
        try:
            import axon_profile_shim
            axon_profile_shim.install()
        except Exception:
            pass
    res = run_bass_kernel_spmd(nc, in_maps, core_ids=list(range(8)), trace=trace)
    if res.exec_time_ns is not None:
        print(f"HW exec time: {res.exec_time_ns} ns", flush=True)

    out = np.empty((B, S, D), dtype=np.float32)
    # b_v is an exact constant output offset: softmax rows sum to 1, so
    # attn @ (V + 1 b_v^T) @ W_o = attn @ V @ W_o + b_v @ W_o.
    bv_off = [b_v[g * DG:(g + 1) * DG] @ W_o[g * DG:(g + 1) * DG, :]
              for g in range(2)]
    full_bias = b_o + bv_off[0] + bv_off[1]
    for b in range(B):
        part = res.results[2 * b]["o_t"] + res.results[2 * b + 1]["o_t"]
        out[b] = part.T + full_bias
    return out
